# revision 10
# baseline (speedup 1.0000x reference)
"""Multi-head attention (RoPE + softmax) Trainium2 kernel, 8 NeuronCores.

Sharding: B=2 batches x 16 heads -> each core owns one batch and 4 heads
(tensor-parallel heads within a batch, data-parallel over batch).
Wq/Wk/Wv are split column-wise (by head), Wo row-wise; the 4 partial
Wo-outputs per batch are summed on the host.

Per-core dataflow (all matmuls in fp32r, 1 cycle/row at N>=256):
  1. QKV projection from x^T (h on partitions):  q^T,k^T [256,S], v [t,260]
     (v gets a ones-column per head -> AV matmul emits softmax denominators)
  2. RoPE on q^T/k^T: partition-rotation via a PE permutation matmul,
     then q*cos + rot(q)*sin on DVE (sign folded into the sin table).
  3. Per head pair (row-tiled, concurrent on PE): scores^T [k,q] =
     K_rope^T.T @ Q_rope^T, exp on ScalarE (mask folded into per-partition
     bias), AV accumulation o^T[65,q] with sums in row 64.
  4. Normalize o^T by broadcasting 1/sums via a tiny selector matmul.
  5. y^T = Wo_local^T.T @ o_norm^T, DMA'd straight from PSUM.

This container's walrus accepts only ONE sync-wait per instruction; Tile
emits many multi-wait instructions, so `_split_multiwait` rewrites them
into chains of single-wait NoOps before compiling.
"""

import os
import sys

for _p in ("/opt/trn_rl_repo", "/root/.axon_site/_ro/trn_rl_repo"):
    if os.path.isdir(_p) and _p not in sys.path:
        sys.path.insert(0, _p)

import numpy as np

import bass_rust
import concourse.bass as bass
import concourse.tile as tile
from concourse import mybir
from concourse.bass_utils import run_bass_kernel_spmd

B, S, H = 2, 2048, 1024
NH, HD = 16, 64
ROPE_BASE = 10000.0
N_CORES = 8
HPC = 4                # heads per core
DL = HPC * HD          # local head dims per core (256)
F32 = mybir.dt.float32
F32R = mybir.dt.float32r
AF = mybir.ActivationFunctionType

KT = S // 128          # 16 key tiles
QT = S // 512          # 4 query tiles of 512
HT = H // 128          # 8 hidden k-tiles


def _split_multiwait(nc):
    """Split multi-wait instructions into single-wait NoOp chains."""
    n_new = 0
    for f in nc.m.functions:
        for b in f.blocks:
            il = b.instructions
            i = 0
            while i < len(il):
                ins = il[i]
                si = getattr(ins, "sync_info", None)
                if si is not None and si.on_wait is not None and len(si.on_wait) > 1:
                    waits = list(si.on_wait)
                    ups = list(si.on_update) if si.on_update else []
                    ins.sync_info = bass_rust.SyncInfo(on_wait=[waits[-1]], on_update=ups)
                    nops = []
                    for j, w in enumerate(waits[:-1]):
                        nop = bass_rust.InstNoOp(
                            name=f"{ins.name}-w{j}",
                            engine=ins.engine,
                            sync_info=bass_rust.SyncInfo(on_wait=[w], on_update=[]),
                            bass_nofuse=True,
                        )
                        nops.append(nop)
                    il[i:i] = nops
                    n_new += len(nops)
                    i += len(nops)
                i += 1
    return n_new


def _emit_body(nc, tc, d):
    """Emit one full forward pass. d = dict of DRAM APs."""
    import contextlib

    with contextlib.ExitStack() as ctx:
        const = ctx.enter_context(tc.tile_pool(name="const", bufs=1))

        cos_sb = const.tile([128, S], F32)
        nc.sync.dma_start(out=cos_sb[:], in_=d["cosT"][:])
        sin_sb = const.tile([128, S], F32)
        nc.sync.dma_start(out=sin_sb[:], in_=d["sinT"][:])
        perm_sb = const.tile([128, 128], F32R)
        nc.sync.dma_start(out=perm_sb[:], in_=d["perm"].bitcast(F32R)[:])
        sel_sb = const.tile([4, 256], F32R)
        nc.sync.dma_start(out=sel_sb[:], in_=d["sel"].bitcast(F32R)[:])
        mb_sb = const.tile([128, KT], F32)
        nc.sync.dma_start(out=mb_sb[:], in_=d["mbias"][:])
        ones_c = const.tile([128, 1], F32)
        nc.vector.memset(ones_c[:], 1.0)
        wqk_sb = const.tile([128, HT, 512], F32R)
        nc.sync.dma_start(
            out=wqk_sb[:],
            in_=d["wqkT"].bitcast(F32R).rearrange("(k p) m -> p k m", p=128)[:],
        )
        wv_sb = const.tile([128, HT, 260], F32R)
        nc.sync.dma_start(
            out=wv_sb[:],
            in_=d["wvT"].bitcast(F32R).rearrange("(k p) m -> p k m", p=128)[:],
        )
        wo_sb = const.tile([128, 2, H], F32R)
        nc.sync.dma_start(
            out=wo_sb[:],
            in_=d["woT"].bitcast(F32R).rearrange("(k p) m -> p k m", p=128)[:],
        )

        xT_r = d["xT"].bitcast(F32R)

        # persistent activations
        actp = ctx.enter_context(tc.tile_pool(name="actp", bufs=1))
        qrope = [actp.tile([128, S], F32R, tag=f"qrope{m}", name=f"qrope{m}") for m in range(2)]
        krope = [actp.tile([128, S], F32R, tag=f"krope{m}", name=f"krope{m}") for m in range(2)]
        v_sb = [actp.tile([128, 260], F32R, tag=f"v{t}", name=f"v{t}") for t in range(KT)]
        o_sb = [actp.tile([128, S], F32, tag=f"o{m}", name=f"o{m}") for m in range(2)]
        sums_sb = actp.tile([4, S], F32, tag="sums", name="sums")

        # ---------------- phase 1: QKV projections -------------------
        with tc.tile_pool(name="p1", bufs=1) as p1, \
             tc.tile_pool(name="xk", bufs=3) as xkp, \
             tc.tile_pool(name="pqk", bufs=1, space="PSUM") as pqkp:
            qkraw = [p1.tile([128, S], F32R, tag=f"qkraw{m}", name=f"qkraw{m}") for m in range(4)]
            for t in range(QT):
                tsl = bass.ts(t, 512)
                ps = [pqkp.tile([128, 512], F32, tag=f"pq{m}", name=f"pq{m}") for m in range(4)]
                for kt in range(HT):
                    xk = xkp.tile([128, 512], F32R, tag="xk")
                    nc.sync.dma_start(out=xk[:], in_=xT_r[kt * 128:(kt + 1) * 128, tsl])
                    for m in range(4):
                        nc.tensor.matmul(
                            ps[m][:], wqk_sb[:, kt, m * 128:(m + 1) * 128], xk[:],
                            start=(kt == 0), stop=(kt == HT - 1),
                        )
                for m in range(4):
                    nc.scalar.copy(out=qkraw[m][:, tsl], in_=ps[m][:])

            # ---- v projection: v [t,260] tiles (x^T slices as weights)
            with tc.tile_pool(name="xk2", bufs=4) as xk2p, \
                 tc.tile_pool(name="pv", bufs=2, space="PSUM") as pvp:
                for t in range(KT):
                    pv = pvp.tile([128, 260], F32, tag="pv")
                    for kt in range(HT):
                        xk2 = xk2p.tile([128, 128], F32R, tag="xk2")
                        nc.sync.dma_start(
                            out=xk2[:],
                            in_=xT_r[kt * 128:(kt + 1) * 128, t * 128:(t + 1) * 128],
                        )
                        nc.tensor.matmul(
                            pv[:], xk2[:], wv_sb[:, kt, :],
                            start=(kt == 0), stop=(kt == HT - 1),
                        )
                    nc.vector.tensor_copy(out=v_sb[t][:], in_=pv[:])
                    for hh in range(HPC):
                        nc.vector.tensor_copy(
                            out=v_sb[t][:, hh * 65 + 64:hh * 65 + 65], in_=ones_c[:])

            # ---------------- phase 2: RoPE ---------------------------
            with tc.tile_pool(name="psh", bufs=3, space="PSUM") as pshp, \
                 tc.tile_pool(name="rtmp", bufs=3) as rtp:
                for m in range(4):
                    dst = qrope[m] if m < 2 else krope[m - 2]
                    for t in range(QT):
                        tsl = bass.ts(t, 512)
                        sh = pshp.tile([128, 512], F32, tag="sh")
                        nc.tensor.matmul(sh[:], perm_sb[:], qkraw[m][:, tsl],
                                         start=True, stop=True)
                        t1 = rtp.tile([128, 512], F32, tag="t1")
                        nc.vector.tensor_mul(t1[:], qkraw[m][:, tsl], cos_sb[:, tsl])
                        t2 = rtp.tile([128, 512], F32, tag="t2")
                        nc.vector.tensor_mul(t2[:], sh[:], sin_sb[:, tsl])
                        nc.vector.tensor_add(dst[:, tsl], t1[:], t2[:])

        # ---------------- phase 3: attention --------------------------
        with tc.tile_pool(name="psc", bufs=2, space="PSUM") as pscp, \
             tc.tile_pool(name="po", bufs=1, space="PSUM") as pop, \
             tc.tile_pool(name="ostg", bufs=2) as ostgp, \
             tc.tile_pool(name="expp", bufs=3) as expp:
            for pair in range(2):           # heads (2*pair, 2*pair+1)
                for qt in range(QT):
                    qsl = bass.ts(qt, 512)
                    oA = pop.tile([65, 512], F32, tag="oA", name="oA")
                    oB = pop.tile([65, 512], F32, tag="oB", name="oB")
                    for kt in range(KT):
                        ksl = bass.ts(kt, 128)
                        stA = pscp.tile([128, 512], F32, tag="stA", name="stA")
                        nc.tensor.matmul(stA[:], krope[pair][0:64, ksl],
                                         qrope[pair][0:64, qsl], start=True, stop=True)
                        stB = pscp.tile([128, 512], F32, tag="stB", name="stB")
                        nc.tensor.matmul(stB[:], krope[pair][64:128, ksl],
                                         qrope[pair][64:128, qsl], start=True, stop=True)
                        eA = expp.tile([128, 512], F32R, tag="eA", name="eA")
                        nc.scalar.activation(eA[:], stA[:], AF.Exp,
                                             bias=mb_sb[:, kt:kt + 1])
                        eB = expp.tile([128, 512], F32R, tag="eB", name="eB")
                        nc.scalar.activation(eB[:], stB[:], AF.Exp,
                                             bias=mb_sb[:, kt:kt + 1])
                        hA, hB = 2 * pair, 2 * pair + 1
                        nc.tensor.matmul(oA[:], v_sb[kt][:, hA * 65:hA * 65 + 65],
                                         eA[:], start=(kt == 0), stop=(kt == KT - 1))
                        nc.tensor.matmul(oB[:], v_sb[kt][:, hB * 65:hB * 65 + 65],
                                         eB[:], start=(kt == 0), stop=(kt == KT - 1))
                    # drain: PSUM -> SBUF staging (DVE), then DMA (can shift
                    # partitions SBUF->SBUF) to assemble o_sb and sums_sb
                    stgA = ostgp.tile([65, 512], F32, tag="stgA", name="stgA")
                    nc.vector.tensor_copy(stgA[:], oA[:])
                    stgB = ostgp.tile([65, 512], F32, tag="stgB", name="stgB")
                    nc.vector.tensor_copy(stgB[:], oB[:])
                    nc.sync.dma_start(out=o_sb[pair][0:64, qsl], in_=stgA[0:64, :])
                    nc.sync.dma_start(out=o_sb[pair][64:128, qsl], in_=stgB[0:64, :])
                    nc.sync.dma_start(out=sums_sb[2 * pair:2 * pair + 1, qsl],
                                      in_=stgA[64:65, :])
                    nc.sync.dma_start(out=sums_sb[2 * pair + 1:2 * pair + 2, qsl],
                                      in_=stgB[64:65, :])

        # ---------------- phase 4: normalize + Wo projection ----------
        with tc.tile_pool(name="nrm", bufs=1) as nrmp, \
             tc.tile_pool(name="pbc", bufs=2, space="PSUM") as pbcp, \
             tc.tile_pool(name="ysbp", bufs=4) as ysbp, \
             tc.tile_pool(name="py", bufs=4, space="PSUM") as pyp:
            rec_f = nrmp.tile([4, S], F32, tag="rec_f")
            nc.vector.reciprocal(rec_f[:], sums_sb[:])
            rec = nrmp.tile([4, S], F32R, tag="rec")
            nc.vector.tensor_copy(rec[:], rec_f[:])
            onorm = [nrmp.tile([128, S], F32R, tag=f"onorm{m}", name=f"onorm{m}") for m in range(2)]
            for m in range(2):
                for qt in range(QT):
                    qsl = bass.ts(qt, 512)
                    bc = pbcp.tile([128, 512], F32, tag="bc")
                    nc.tensor.matmul(bc[:], sel_sb[:, m * 128:(m + 1) * 128],
                                     rec[:, qsl], start=True, stop=True)
                    nc.vector.tensor_mul(onorm[m][:, qsl], o_sb[m][:, qsl], bc[:])
            for mo in range(HT):
                for t in range(QT):
                    tsl = bass.ts(t, 512)
                    yp = pyp.tile([128, 512], F32, tag="yp", name="yp")
                    for k2 in range(2):
                        nc.tensor.matmul(yp[:], wo_sb[:, k2, mo * 128:(mo + 1) * 128],
                                         onorm[k2][:, tsl],
                                         start=(k2 == 0), stop=(k2 == 1))
                    ysb = ysbp.tile([128, 512], F32, tag="ysb", name="ysb")
                    nc.vector.tensor_copy(ysb[:], yp[:])
                    nc.sync.dma_start(out=d["yT"][mo * 128:(mo + 1) * 128, tsl],
                                      in_=ysb[:])


def build(reps=1, split=True):
    nc = bass.Bass("TRN2", target_bir_lowering=False, debug=False,
                   num_devices=N_CORES)
    d = {
        "xT": nc.dram_tensor("xT", [H, S], F32, kind="ExternalInput"),
        "wqkT": nc.dram_tensor("wqkT", [H, 512], F32, kind="ExternalInput"),
        "wvT": nc.dram_tensor("wvT", [H, 260], F32, kind="ExternalInput"),
        "woT": nc.dram_tensor("woT", [DL, H], F32, kind="ExternalInput"),
        "cosT": nc.dram_tensor("cosT", [128, S], F32, kind="ExternalInput"),
        "sinT": nc.dram_tensor("sinT", [128, S], F32, kind="ExternalInput"),
        "perm": nc.dram_tensor("perm", [128, 128], F32, kind="ExternalInput"),
        "sel": nc.dram_tensor("sel", [4, 256], F32, kind="ExternalInput"),
        "mbias": nc.dram_tensor("mbias", [128, KT], F32, kind="ExternalInput"),
        "yT": nc.dram_tensor("yT", [H, S], F32, kind="ExternalOutput"),
    }
    with tile.TileContext(nc) as tc:
        if reps == 1:
            _emit_body(nc, tc, d)
        else:
            with tc.For_i(0, reps, 1):
                _emit_body(nc, tc, d)
    if split:
        _split_multiwait(nc)
    return nc


def host_inputs(x, attention_mask, Wq, Wk, Wv, Wo):
    """Build the 8 per-core input maps (numpy only)."""
    x = np.asarray(x, dtype=np.float32)
    attention_mask = np.asarray(attention_mask, dtype=np.float32)
    Wq = np.asarray(Wq, dtype=np.float32)
    Wk = np.asarray(Wk, dtype=np.float32)
    Wv = np.asarray(Wv, dtype=np.float32)
    Wo = np.asarray(Wo, dtype=np.float32)

    xT = [np.ascontiguousarray(x[b].T) for b in range(B)]

    p = np.arange(128)
    dd = p % HD
    inv = ROPE_BASE ** (-(dd % 32).astype(np.float32) / 32.0)
    s = np.arange(S, dtype=np.float32)
    ang = inv[:, None] * s[None, :]
    cosT = np.cos(ang).astype(np.float32)
    sinT = (np.where(dd < 32, -1.0, 1.0)[:, None] * np.sin(ang)).astype(np.float32)

    perm = np.zeros((128, 128), dtype=np.float32)
    for m in range(128):
        head, d_ = m // HD, m % HD
        perm[head * HD + (d_ + 32) % HD, m] = 1.0

    sel = np.zeros((4, 256), dtype=np.float32)
    for m in range(256):
        sel[m // HD, m] = 1.0

    mb = [(-1e30 * (1.0 - attention_mask[b])).astype(np.float32)
          .reshape(KT, 128).T.copy() for b in range(B)]

    in_maps = []
    for c in range(N_CORES):
        b, hq = c // 4, (c % 4) * HPC
        d0 = hq * HD
        wqkT = np.ascontiguousarray(
            np.concatenate([Wq[d0:d0 + DL] * (HD ** -0.5), Wk[d0:d0 + DL]], axis=0).T
        )
        wvT = np.zeros((H, 260), dtype=np.float32)
        for hh in range(HPC):
            wvT[:, hh * 65:hh * 65 + 64] = Wv[d0 + hh * HD:d0 + (hh + 1) * HD].T
        woT = np.ascontiguousarray(Wo[:, d0:d0 + DL].T)
        in_maps.append({
            "xT": xT[b], "wqkT": wqkT, "wvT": wvT, "woT": woT,
            "cosT": cosT, "sinT": sinT, "perm": perm, "sel": sel, "mbias": mb[b],
        })
    return in_maps


def gather_output(results):
    y = np.zeros((B, S, H), dtype=np.float32)
    for c in range(N_CORES):
        y[c // 4] += results[c]["yT"].T
    return y


_nc_cache = {}


def kernel(x, attention_mask, Wq, Wk, Wv, Wo):
    if "nc" not in _nc_cache:
        _nc_cache["nc"] = build(reps=1)
    nc = _nc_cache["nc"]
    in_maps = host_inputs(x, attention_mask, Wq, Wk, Wv, Wo)
    res = run_bass_kernel_spmd(nc, in_maps, list(range(N_CORES)), trace=False)
    return gather_output(res.results)


# revision 20
# speedup vs baseline: 17.7197x; 17.7197x over previous
"""Multi-head attention (RoPE + softmax) Trainium2 kernel, 8 NeuronCores.

Sharding: B=2 batches x 16 heads -> each core owns one batch and 4 heads
(tensor-parallel heads within a batch, data-parallel over batch).
Wq/Wk/Wv are split column-wise (by head), Wo row-wise; the 4 partial
Wo-outputs per batch are summed on the host.

Per-core dataflow:
  1. QKV projection from x^T (h on partitions), fp32r accumulation
     chains: q^T,k^T [256,S] and v [t,260] (v carries a ones-column per
     head so the AV matmul emits softmax denominators for free).
  2. RoPE on q^T/k^T: partition-rotation via a PE permutation matmul,
     then q*cos + rot(q)*sin on DVE (sign folded into the sin table).
  3. Assembled per-head [65,S] fp16 Q'/K' tiles: row 64 of K' holds the
     additive attention-mask bias, row 64 of Q' holds ones, so
     scores = K'.T @ Q' includes the mask via the contraction and the
     exp needs no per-partition bias -> one ACTIVATE per [128,1024]
     PSUM block (fp16 matmuls use explicit LDWEIGHTS, which the PE
     pipelines; fp32r self-loading matmuls cannot).
  4. AV accumulation o^T[65,q] in fp16 with sums in row 64; normalize
     via a tiny selector matmul broadcasting 1/sums.
  5. y^T = Wo_local^T.T @ o_norm^T (fp32r chains), staged to SBUF, DMA'd.

This container's walrus accepts only ONE sync-wait per instruction; Tile
emits many multi-wait instructions, so `_split_multiwait` rewrites them
into chains of single-wait NoOps before compiling.
"""

import os
import sys

for _p in ("/opt/trn_rl_repo", "/root/.axon_site/_ro/trn_rl_repo"):
    if os.path.isdir(_p) and _p not in sys.path:
        sys.path.insert(0, _p)

import contextlib

import numpy as np

import bass_rust
import concourse.bass as bass
import concourse.tile as tile
from concourse import mybir
from concourse.bass_utils import run_bass_kernel_spmd

B, S, H = 2, 2048, 1024
NH, HD = 16, 64
ROPE_BASE = 10000.0
N_CORES = 8
HPC = 4                # heads per core
DL = HPC * HD          # local head dims per core (256)
F32 = mybir.dt.float32
F32R = mybir.dt.float32r
FP16 = mybir.dt.float16
AF = mybir.ActivationFunctionType

KT = S // 128          # 16 key tiles
QT = S // 512          # 4 query tiles of 512
HT = H // 128          # 8 hidden k-tiles

SC_DT = FP16           # scores/exp/V/AV dtype
PJ_DT = FP16           # projection (x, Wq/Wk/Wv/Wo) dtype
MASK_NEG = -60000.0    # large-negative mask bias, finite in fp16

_np_dt = {F32R: np.float32, F32: np.float32, FP16: np.float16}


def _split_multiwait(nc):
    """Split multi-wait instructions into single-wait NoOp chains."""
    n_new = 0
    for f in nc.m.functions:
        for b in f.blocks:
            il = b.instructions
            i = 0
            while i < len(il):
                ins = il[i]
                si = getattr(ins, "sync_info", None)
                if si is not None and si.on_wait is not None and len(si.on_wait) > 1:
                    waits = list(si.on_wait)
                    ups = list(si.on_update) if si.on_update else []
                    ins.sync_info = bass_rust.SyncInfo(on_wait=[waits[-1]], on_update=ups)
                    nops = []
                    for j, w in enumerate(waits[:-1]):
                        nop = bass_rust.InstNoOp(
                            name=f"{ins.name}-w{j}",
                            engine=ins.engine,
                            sync_info=bass_rust.SyncInfo(on_wait=[w], on_update=[]),
                            bass_nofuse=True,
                        )
                        nops.append(nop)
                    il[i:i] = nops
                    n_new += len(nops)
                    i += len(nops)
                i += 1
    return n_new


def _emit_body(nc, tc, d, phases=4):
    with contextlib.ExitStack() as ctx:
        const = ctx.enter_context(tc.tile_pool(name="const", bufs=1))

        cos_sb = const.tile([128, S], F32, name="cos_sb")
        nc.sync.dma_start(out=cos_sb[:], in_=d["cosT"][:])
        sin_sb = const.tile([128, S], F32, name="sin_sb")
        nc.sync.dma_start(out=sin_sb[:], in_=d["sinT"][:])
        perm_sb = const.tile([128, 128], SC_DT, name="perm_sb")
        nc.sync.dma_start(out=perm_sb[:], in_=d["perm"][:])
        sel_sb = const.tile([4, 256], F32R, name="sel_sb")
        nc.sync.dma_start(out=sel_sb[:], in_=d["sel"].bitcast(F32R)[:])
        ones_c = const.tile([128, 1], F32, name="ones_c")
        nc.vector.memset(ones_c[:], 1.0)
        wqkT = d["wqkT"].bitcast(F32R) if PJ_DT == F32R else d["wqkT"]
        wqk_sb = const.tile([128, HT, 512], PJ_DT, name="wqk_sb")
        nc.sync.dma_start(out=wqk_sb[:], in_=wqkT.rearrange("(k p) m -> p k m", p=128)[:])
        wvT = d["wvT"].bitcast(F32R) if PJ_DT == F32R else d["wvT"]
        wv_sb = const.tile([128, HT, 260], PJ_DT, name="wv_sb")
        nc.sync.dma_start(out=wv_sb[:], in_=wvT.rearrange("(k p) m -> p k m", p=128)[:])
        woT = d["woT"].bitcast(F32R) if PJ_DT == F32R else d["woT"]
        wo_sb = const.tile([128, 2, H], PJ_DT, name="wo_sb")
        nc.sync.dma_start(out=wo_sb[:], in_=woT.rearrange("(k p) m -> p k m", p=128)[:])

        xT_r = d["xT"].bitcast(F32R) if PJ_DT == F32R else d["xT"]
        x_sb = const.tile([128, HT, S], PJ_DT, name="x_sb")
        nc.sync.dma_start(out=x_sb[:], in_=xT_r.rearrange("(k p) m -> p k m", p=128)[:])

        # persistent activations
        actp = ctx.enter_context(tc.tile_pool(name="actp", bufs=1))
        qr_a = [actp.tile([65, S], SC_DT, name=f"qr_a{h}") for h in range(HPC)]
        kr_a = [actp.tile([65, S], SC_DT, name=f"kr_a{h}") for h in range(HPC)]
        v_sb = [actp.tile([128, 260], SC_DT, name=f"v{t}") for t in range(KT)]
        o_sb = [actp.tile([128, S], F32, name=f"o{m}") for m in range(2)]
        sums_sb = actp.tile([4, S], F32, name="sums")

        # ---------------- phase 1: QKV projections -------------------
        with tc.tile_pool(name="p1", bufs=1) as p1, \
             tc.tile_pool(name="pqk", bufs=1, space="PSUM") as pqkp:
            qkraw = [p1.tile([128, S], SC_DT, name=f"qkraw{m}") for m in range(4)]
            for t in range(QT):
                tsl = bass.ts(t, 512)
                ps = [pqkp.tile([128, 512], F32, tag=f"pq{m}", name=f"pq{m}")
                      for m in range(4)]
                for kt in range(HT):
                    for m in range(4):
                        nc.tensor.matmul(
                            ps[m][:], wqk_sb[:, kt, m * 128:(m + 1) * 128],
                            x_sb[:, kt, tsl],
                            start=(kt == 0), stop=(kt == HT - 1),
                        )
                for m in range(4):
                    nc.vector.tensor_copy(out=qkraw[m][:, tsl], in_=ps[m][:])

            # ---- v projection: v [t,260] tiles (x^T slices as weights)
            with tc.tile_pool(name="pv", bufs=2, space="PSUM") as pvp:
                for t in range(KT):
                    pv = pvp.tile([128, 260], F32, tag="pv", name="pv")
                    for kt in range(HT):
                        nc.tensor.matmul(
                            pv[:], x_sb[:, kt, t * 128:(t + 1) * 128], wv_sb[:, kt, :],
                            start=(kt == 0), stop=(kt == HT - 1),
                        )
                    nc.vector.tensor_copy(out=v_sb[t][:], in_=pv[:])
                    for hh in range(HPC):
                        nc.vector.tensor_copy(
                            out=v_sb[t][:, hh * 65 + 64:hh * 65 + 65], in_=ones_c[:])

            if phases < 2:
                return
            # ---------------- phase 2: RoPE + assembly ----------------
            with tc.tile_pool(name="psh", bufs=3, space="PSUM") as pshp, \
                 tc.tile_pool(name="rope", bufs=1) as ropep, \
                 tc.tile_pool(name="rtmp", bufs=3) as rtp:
                qrope = [ropep.tile([128, S], SC_DT, name=f"qrope{m}") for m in range(2)]
                krope = [ropep.tile([128, S], SC_DT, name=f"krope{m}") for m in range(2)]
                for m in range(4):
                    dst = qrope[m] if m < 2 else krope[m - 2]
                    for t in range(QT):
                        tsl = bass.ts(t, 512)
                        sh = pshp.tile([128, 512], F32, tag="sh", name="sh")
                        nc.tensor.matmul(sh[:], perm_sb[:], qkraw[m][:, tsl],
                                         start=True, stop=True)
                        t1 = rtp.tile([128, 512], F32, tag="t1", name="t1")
                        nc.vector.tensor_mul(t1[:], qkraw[m][:, tsl], cos_sb[:, tsl])
                        t2 = rtp.tile([128, 512], F32, tag="t2", name="t2")
                        nc.vector.tensor_mul(t2[:], sh[:], sin_sb[:, tsl])
                        nc.vector.tensor_add(dst[:, tsl], t1[:], t2[:])
                # assemble per-head [65,S] tiles: rows 0-63 = head slice,
                # row 64 = ones (Q) / mask bias (K). SBUF->SBUF DMA can
                # shift partitions freely.
                for h in range(HPC):
                    nc.scalar.dma_start(out=qr_a[h][0:64, :],
                                        in_=qrope[h // 2][(h % 2) * 64:(h % 2) * 64 + 64, :])
                    nc.scalar.dma_start(out=qr_a[h][64:65, :], in_=d["ones_row"][:])
                    nc.scalar.dma_start(out=kr_a[h][0:64, :],
                                        in_=krope[h // 2][(h % 2) * 64:(h % 2) * 64 + 64, :])
                    nc.scalar.dma_start(out=kr_a[h][64:65, :], in_=d["mrow"][:])

        if phases < 3:
            return
        # ---------------- phase 3: attention --------------------------
        with tc.tile_pool(name="psc", bufs=2, space="PSUM") as pscp, \
             tc.tile_pool(name="po", bufs=2, space="PSUM") as pop, \
             tc.tile_pool(name="ostg", bufs=2) as ostgp, \
             tc.tile_pool(name="expp", bufs=4) as expp:
            for h in range(HPC):
                for qb in range(2):            # 1024-wide query blocks
                    q0 = qb * 1024
                    oP = [pop.tile([65, 512], F32, tag=f"o{j}", name=f"o{j}")
                          for j in range(2)]
                    for kt in range(KT):
                        st = pscp.tile([128, 1024], F32, tag="st", name="st")
                        for j in range(2):
                            nc.tensor.matmul(
                                st[:, j * 512:(j + 1) * 512],
                                kr_a[h][:, kt * 128:(kt + 1) * 128],
                                qr_a[h][:, q0 + j * 512:q0 + (j + 1) * 512],
                                start=True, stop=True)
                        e = expp.tile([128, 1024], SC_DT, tag="e", name="e")
                        nc.scalar.activation(e[:], st[:], AF.Exp)
                        for j in range(2):
                            nc.tensor.matmul(
                                oP[j][:], v_sb[kt][:, h * 65:h * 65 + 65],
                                e[:, j * 512:(j + 1) * 512],
                                start=(kt == 0), stop=(kt == KT - 1))
                    for j in range(2):
                        qsl = bass.ds(q0 + j * 512, 512)
                        stg = ostgp.tile([65, 512], F32, tag=f"stg{j}", name=f"stg{j}")
                        nc.vector.tensor_copy(stg[:], oP[j][:])
                        nc.scalar.dma_start(
                            out=o_sb[h // 2][(h % 2) * 64:(h % 2) * 64 + 64, qsl],
                            in_=stg[0:64, :])
                        nc.scalar.dma_start(out=sums_sb[h:h + 1, qsl], in_=stg[64:65, :])

        if phases < 4:
            return
        # ---------------- phase 4: normalize + Wo projection ----------
        with tc.tile_pool(name="nrm", bufs=1) as nrmp, \
             tc.tile_pool(name="pbc", bufs=2, space="PSUM") as pbcp, \
             tc.tile_pool(name="ysbp", bufs=4) as ysbp, \
             tc.tile_pool(name="py", bufs=4, space="PSUM") as pyp:
            rec_f = nrmp.tile([4, S], F32, name="rec_f")
            nc.vector.reciprocal(rec_f[:], sums_sb[:])
            rec = nrmp.tile([4, S], F32R, name="rec")
            nc.vector.tensor_copy(rec[:], rec_f[:])
            onorm = [nrmp.tile([128, S], PJ_DT, name=f"onorm{m}") for m in range(2)]
            for m in range(2):
                for qt in range(QT):
                    qsl = bass.ts(qt, 512)
                    bc = pbcp.tile([128, 512], F32, tag="bc", name="bc")
                    nc.tensor.matmul(bc[:], sel_sb[:, m * 128:(m + 1) * 128],
                                     rec[:, qsl], start=True, stop=True)
                    nc.vector.tensor_mul(onorm[m][:, qsl], o_sb[m][:, qsl], bc[:])
            for mo in range(HT):
                for t in range(QT):
                    tsl = bass.ts(t, 512)
                    yp = pyp.tile([128, 512], F32, tag="yp", name="yp")
                    for k2 in range(2):
                        nc.tensor.matmul(yp[:], wo_sb[:, k2, mo * 128:(mo + 1) * 128],
                                         onorm[k2][:, tsl],
                                         start=(k2 == 0), stop=(k2 == 1))
                    ysb = ysbp.tile([128, 512], F32, tag="ysb", name="ysb")
                    nc.vector.tensor_copy(ysb[:], yp[:])
                    nc.scalar.dma_start(out=d["yT"][mo * 128:(mo + 1) * 128, tsl],
                                        in_=ysb[:])


def build(reps=1, split=True, phases=4):
    nc = bass.Bass("TRN2", target_bir_lowering=False, debug=False,
                   num_devices=N_CORES)
    pj = F32 if PJ_DT == F32R else PJ_DT
    sc = F32 if SC_DT == F32R else SC_DT
    d = {
        "xT": nc.dram_tensor("xT", [H, S], pj, kind="ExternalInput"),
        "wqkT": nc.dram_tensor("wqkT", [H, 512], pj, kind="ExternalInput"),
        "wvT": nc.dram_tensor("wvT", [H, 260], pj, kind="ExternalInput"),
        "woT": nc.dram_tensor("woT", [DL, H], pj, kind="ExternalInput"),
        "cosT": nc.dram_tensor("cosT", [128, S], F32, kind="ExternalInput"),
        "sinT": nc.dram_tensor("sinT", [128, S], F32, kind="ExternalInput"),
        "perm": nc.dram_tensor("perm", [128, 128], sc, kind="ExternalInput"),
        "sel": nc.dram_tensor("sel", [4, 256], F32, kind="ExternalInput"),
        "mrow": nc.dram_tensor("mrow", [1, S], sc, kind="ExternalInput"),
        "ones_row": nc.dram_tensor("ones_row", [1, S], sc, kind="ExternalInput"),
        "yT": nc.dram_tensor("yT", [H, S], F32, kind="ExternalOutput"),
    }
    with tile.TileContext(nc) as tc:
        if reps == 1:
            _emit_body(nc, tc, d, phases)
        else:
            with tc.For_i(0, reps, 1):
                _emit_body(nc, tc, d, phases)
    if split:
        _split_multiwait(nc)
    return nc


def host_inputs(x, attention_mask, Wq, Wk, Wv, Wo):
    """Build the 8 per-core input maps (numpy only)."""
    x = np.asarray(x, dtype=np.float32)
    attention_mask = np.asarray(attention_mask, dtype=np.float32)
    Wq = np.asarray(Wq, dtype=np.float32)
    Wk = np.asarray(Wk, dtype=np.float32)
    Wv = np.asarray(Wv, dtype=np.float32)
    Wo = np.asarray(Wo, dtype=np.float32)

    pj_np = _np_dt[PJ_DT]
    sc_np = _np_dt[SC_DT]

    xT = [np.ascontiguousarray(x[b].T).astype(pj_np) for b in range(B)]

    p = np.arange(128)
    dd = p % HD
    inv = ROPE_BASE ** (-(dd % 32).astype(np.float32) / 32.0)
    s = np.arange(S, dtype=np.float32)
    ang = inv[:, None] * s[None, :]
    cosT = np.cos(ang).astype(np.float32)
    sinT = (np.where(dd < 32, -1.0, 1.0)[:, None] * np.sin(ang)).astype(np.float32)

    perm = np.zeros((128, 128), dtype=sc_np)
    for m in range(128):
        head, d_ = m // HD, m % HD
        perm[head * HD + (d_ + 32) % HD, m] = 1.0

    sel = np.zeros((4, 256), dtype=np.float32)
    for m in range(256):
        sel[m // HD, m] = 1.0

    ones_row = np.ones((1, S), dtype=sc_np)
    mrow = [(MASK_NEG * (1.0 - attention_mask[b])).astype(sc_np).reshape(1, S)
            for b in range(B)]

    in_maps = []
    for c in range(N_CORES):
        b, hq = c // 4, (c % 4) * HPC
        d0 = hq * HD
        wqkT = np.ascontiguousarray(
            np.concatenate([Wq[d0:d0 + DL] * (HD ** -0.5), Wk[d0:d0 + DL]], axis=0).T
        ).astype(pj_np)
        wvT = np.zeros((H, 260), dtype=pj_np)
        for hh in range(HPC):
            wvT[:, hh * 65:hh * 65 + 64] = Wv[d0 + hh * HD:d0 + (hh + 1) * HD].T
        woT = np.ascontiguousarray(Wo[:, d0:d0 + DL].T).astype(pj_np)
        in_maps.append({
            "xT": xT[b], "wqkT": wqkT, "wvT": wvT, "woT": woT,
            "cosT": cosT, "sinT": sinT, "perm": perm, "sel": sel,
            "mrow": mrow[b], "ones_row": ones_row,
        })
    return in_maps


def gather_output(results):
    y = np.zeros((B, S, H), dtype=np.float32)
    for c in range(N_CORES):
        y[c // 4] += results[c]["yT"].T
    return y


_nc_cache = {}


def kernel(x, attention_mask, Wq, Wk, Wv, Wo):
    if "nc" not in _nc_cache:
        _nc_cache["nc"] = build(reps=1)
    nc = _nc_cache["nc"]
    in_maps = host_inputs(x, attention_mask, Wq, Wk, Wv, Wo)
    res = run_bass_kernel_spmd(nc, in_maps, list(range(N_CORES)), trace=False)
    return gather_output(res.results)


# revision 22
# speedup vs baseline: 17.9994x; 1.0158x over previous
"""Multi-head attention (RoPE + softmax) Trainium2 kernel, 8 NeuronCores.

Sharding: B=2 batches x 16 heads -> each core owns one batch and 4 heads
(tensor-parallel heads within a batch, data-parallel over batch).
Wq/Wk/Wv are split column-wise (by head), Wo row-wise; the 4 partial
Wo-outputs per batch are summed on the host.

Per-core dataflow:
  1. QKV projection from x^T (h on partitions, resident in SBUF), fp16
     accumulation chains: q^T,k^T [256,S] and v [t,260] (v carries a
     ones-column per head so AV emits softmax denominators for free).
  2. RoPE on q^T/k^T: partition-rotation via a PE permutation matmul,
     then q*cos + rot(q)*sin on DVE (sign folded into the sin table).
  3. Assembled per-head [65,S] fp16 Q'/K' tiles: row 64 of K' holds the
     additive attention-mask bias, row 64 of Q' holds ones, so
     scores = K'.T @ Q' includes the mask via the contraction and the
     exp needs no per-partition bias -> one ACTIVATE per [128,1024]
     PSUM block (fp16 matmuls use explicit LDWEIGHTS, which the PE
     pipelines; fp32r self-loading matmuls cannot).
  4. AV accumulation o^T[65,q] in fp16 with sums in row 64; normalize
     via a tiny selector matmul broadcasting 1/sums.
  5. y^T = Wo_local^T.T @ o_norm^T (fp16 chains), staged to SBUF, DMA'd.

This container's walrus accepts only ONE sync-wait per instruction; Tile
emits many multi-wait instructions, so `_split_multiwait` rewrites them
into chains of single-wait NoOps before compiling.
"""

import os
import sys

for _p in ("/opt/trn_rl_repo", "/root/.axon_site/_ro/trn_rl_repo"):
    if os.path.isdir(_p) and _p not in sys.path:
        sys.path.insert(0, _p)

import contextlib

import numpy as np

import bass_rust
import concourse.bass as bass
import concourse.tile as tile
from concourse import mybir
from concourse.bass_utils import run_bass_kernel_spmd

B, S, H = 2, 2048, 1024
NH, HD = 16, 64
ROPE_BASE = 10000.0
N_CORES = 8
HPC = 4                # heads per core
DL = HPC * HD          # local head dims per core (256)
F32 = mybir.dt.float32
F32R = mybir.dt.float32r
FP16 = mybir.dt.float16
AF = mybir.ActivationFunctionType

KT = S // 128          # 16 key tiles
QT = S // 512          # 4 query tiles of 512
HT = H // 128          # 8 hidden k-tiles

SC_DT = FP16           # scores/exp/V/AV dtype
PJ_DT = FP16           # projection (x, Wq/Wk/Wv/Wo) dtype
MASK_NEG = -60000.0    # large-negative mask bias, finite in fp16

_np_dt = {F32R: np.float32, F32: np.float32, FP16: np.float16}


def _split_multiwait(nc):
    """Split multi-wait instructions into single-wait NoOp chains."""
    n_new = 0
    for f in nc.m.functions:
        for b in f.blocks:
            il = b.instructions
            i = 0
            while i < len(il):
                ins = il[i]
                si = getattr(ins, "sync_info", None)
                if si is not None and si.on_wait is not None and len(si.on_wait) > 1:
                    waits = list(si.on_wait)
                    ups = list(si.on_update) if si.on_update else []
                    ins.sync_info = bass_rust.SyncInfo(on_wait=[waits[-1]], on_update=ups)
                    nops = []
                    for j, w in enumerate(waits[:-1]):
                        nop = bass_rust.InstNoOp(
                            name=f"{ins.name}-w{j}",
                            engine=ins.engine,
                            sync_info=bass_rust.SyncInfo(on_wait=[w], on_update=[]),
                            bass_nofuse=True,
                        )
                        nops.append(nop)
                    il[i:i] = nops
                    n_new += len(nops)
                    i += len(nops)
                i += 1
    return n_new


def _emit_body(nc, tc, d, phases=4):
    with contextlib.ExitStack() as ctx:
        const = ctx.enter_context(tc.tile_pool(name="const", bufs=1))

        cos_sb = const.tile([128, S], F32, name="cos_sb")
        nc.sync.dma_start(out=cos_sb[:], in_=d["cosT"][:])
        sin_sb = const.tile([128, S], F32, name="sin_sb")
        nc.sync.dma_start(out=sin_sb[:], in_=d["sinT"][:])
        perm_sb = const.tile([128, 128], SC_DT, name="perm_sb")
        nc.sync.dma_start(out=perm_sb[:], in_=d["perm"][:])
        sel_sb = const.tile([4, 256], F32R, name="sel_sb")
        nc.sync.dma_start(out=sel_sb[:], in_=d["sel"].bitcast(F32R)[:])
        ones_c = const.tile([128, 1], F32, name="ones_c")
        nc.vector.memset(ones_c[:], 1.0)
        wqkT = d["wqkT"].bitcast(F32R) if PJ_DT == F32R else d["wqkT"]
        wqk_sb = const.tile([128, HT, 512], PJ_DT, name="wqk_sb")
        nc.sync.dma_start(out=wqk_sb[:], in_=wqkT.rearrange("(k p) m -> p k m", p=128)[:])
        wvT = d["wvT"].bitcast(F32R) if PJ_DT == F32R else d["wvT"]
        wv_sb = const.tile([128, HT, 260], PJ_DT, name="wv_sb")
        nc.sync.dma_start(out=wv_sb[:], in_=wvT.rearrange("(k p) m -> p k m", p=128)[:])
        woT = d["woT"].bitcast(F32R) if PJ_DT == F32R else d["woT"]
        wo_sb = const.tile([128, 2, H], PJ_DT, name="wo_sb")
        nc.sync.dma_start(out=wo_sb[:], in_=woT.rearrange("(k p) m -> p k m", p=128)[:])

        xT_r = d["xT"].bitcast(F32R) if PJ_DT == F32R else d["xT"]
        x_sb = const.tile([128, HT, S], PJ_DT, name="x_sb")
        nc.sync.dma_start(out=x_sb[:], in_=xT_r.rearrange("(k p) m -> p k m", p=128)[:])

        # persistent activations
        actp = ctx.enter_context(tc.tile_pool(name="actp", bufs=1))
        qr_a = [actp.tile([65, S], SC_DT, name=f"qr_a{h}") for h in range(HPC)]
        kr_a = [actp.tile([65, S], SC_DT, name=f"kr_a{h}") for h in range(HPC)]
        v_sb = [actp.tile([128, 260], SC_DT, name=f"v{t}") for t in range(KT)]
        o_sb = [actp.tile([128, S], F32, name=f"o{m}") for m in range(2)]
        sums_sb = actp.tile([4, S], F32, name="sums")

        # ---------------- phase 1: QKV projections -------------------
        with tc.tile_pool(name="p1", bufs=1) as p1:
            qkraw = [p1.tile([128, S], SC_DT, name=f"qkraw{m}") for m in range(4)]
            with tc.tile_pool(name="pqk", bufs=2, space="PSUM") as pqkp:
                for t in range(QT):
                    tsl = bass.ts(t, 512)
                    ps = [pqkp.tile([128, 512], F32, tag=f"pq{m}", name=f"pq{m}")
                          for m in range(4)]
                    for kt in range(HT):
                        for m in range(4):
                            nc.tensor.matmul(
                                ps[m][:], wqk_sb[:, kt, m * 128:(m + 1) * 128],
                                x_sb[:, kt, tsl],
                                start=(kt == 0), stop=(kt == HT - 1),
                            )
                    for m in range(4):
                        nc.vector.tensor_copy(out=qkraw[m][:, tsl], in_=ps[m][:])

            # ---- v projection: v [t,260] tiles (x^T slices as weights)
            with tc.tile_pool(name="pv", bufs=3, space="PSUM") as pvp:
                for t in range(KT):
                    pv = pvp.tile([128, 260], F32, tag="pv", name="pv")
                    for kt in range(HT):
                        nc.tensor.matmul(
                            pv[:], x_sb[:, kt, t * 128:(t + 1) * 128], wv_sb[:, kt, :],
                            start=(kt == 0), stop=(kt == HT - 1),
                        )
                    nc.vector.tensor_copy(out=v_sb[t][:], in_=pv[:])
                    for hh in range(HPC):
                        nc.vector.tensor_copy(
                            out=v_sb[t][:, hh * 65 + 64:hh * 65 + 65], in_=ones_c[:])

            if phases < 2:
                return
            # ---------------- phase 2: RoPE + assembly ----------------
            with tc.tile_pool(name="psh", bufs=3, space="PSUM") as pshp, \
                 tc.tile_pool(name="rope", bufs=1) as ropep, \
                 tc.tile_pool(name="rtmp", bufs=3) as rtp:
                qrope = [ropep.tile([128, S], SC_DT, name=f"qrope{m}") for m in range(2)]
                krope = [ropep.tile([128, S], SC_DT, name=f"krope{m}") for m in range(2)]
                for m in range(4):
                    dst = qrope[m] if m < 2 else krope[m - 2]
                    for t in range(QT):
                        tsl = bass.ts(t, 512)
                        sh = pshp.tile([128, 512], F32, tag="sh", name="sh")
                        nc.tensor.matmul(sh[:], perm_sb[:], qkraw[m][:, tsl],
                                         start=True, stop=True)
                        t1 = rtp.tile([128, 512], F32, tag="t1", name="t1")
                        nc.vector.tensor_mul(t1[:], qkraw[m][:, tsl], cos_sb[:, tsl])
                        t2 = rtp.tile([128, 512], F32, tag="t2", name="t2")
                        nc.vector.tensor_mul(t2[:], sh[:], sin_sb[:, tsl])
                        nc.vector.tensor_add(dst[:, tsl], t1[:], t2[:])
                # assemble per-head [65,S] tiles: rows 0-63 = head slice,
                # row 64 = ones (Q) / mask bias (K). SBUF->SBUF DMA can
                # shift partitions freely.
                for h in range(HPC):
                    nc.scalar.dma_start(out=qr_a[h][0:64, :],
                                        in_=qrope[h // 2][(h % 2) * 64:(h % 2) * 64 + 64, :])
                    nc.scalar.dma_start(out=qr_a[h][64:65, :], in_=d["ones_row"][:])
                    nc.scalar.dma_start(out=kr_a[h][0:64, :],
                                        in_=krope[h // 2][(h % 2) * 64:(h % 2) * 64 + 64, :])
                    nc.scalar.dma_start(out=kr_a[h][64:65, :], in_=d["mrow"][:])

        if phases < 3:
            return
        # ---------------- phase 3: attention --------------------------
        with tc.tile_pool(name="psc", bufs=2, space="PSUM") as pscp, \
             tc.tile_pool(name="po", bufs=2, space="PSUM") as pop, \
             tc.tile_pool(name="ostg", bufs=2) as ostgp, \
             tc.tile_pool(name="expp", bufs=4) as expp:
            for h in range(HPC):
                for qb in range(2):            # 1024-wide query blocks
                    q0 = qb * 1024
                    oP = [pop.tile([65, 512], F32, tag=f"o{j}", name=f"o{j}")
                          for j in range(2)]
                    for kt in range(KT):
                        st = pscp.tile([128, 1024], F32, tag="st", name="st")
                        for j in range(2):
                            nc.tensor.matmul(
                                st[:, j * 512:(j + 1) * 512],
                                kr_a[h][:, kt * 128:(kt + 1) * 128],
                                qr_a[h][:, q0 + j * 512:q0 + (j + 1) * 512],
                                start=True, stop=True)
                        e = expp.tile([128, 1024], SC_DT, tag="e", name="e")
                        nc.scalar.activation(e[:], st[:], AF.Exp)
                        for j in range(2):
                            nc.tensor.matmul(
                                oP[j][:], v_sb[kt][:, h * 65:h * 65 + 65],
                                e[:, j * 512:(j + 1) * 512],
                                start=(kt == 0), stop=(kt == KT - 1))
                    for j in range(2):
                        qsl = bass.ds(q0 + j * 512, 512)
                        stg = ostgp.tile([65, 512], F32, tag=f"stg{j}", name=f"stg{j}")
                        nc.vector.tensor_copy(stg[:], oP[j][:])
                        nc.scalar.dma_start(
                            out=o_sb[h // 2][(h % 2) * 64:(h % 2) * 64 + 64, qsl],
                            in_=stg[0:64, :])
                        nc.scalar.dma_start(out=sums_sb[h:h + 1, qsl], in_=stg[64:65, :])

        if phases < 4:
            return
        # ---------------- phase 4: normalize + Wo projection ----------
        with tc.tile_pool(name="nrm", bufs=1) as nrmp, \
             tc.tile_pool(name="pbc", bufs=2, space="PSUM") as pbcp, \
             tc.tile_pool(name="ysbp", bufs=4) as ysbp, \
             tc.tile_pool(name="py", bufs=4, space="PSUM") as pyp:
            rec_f = nrmp.tile([4, S], F32, name="rec_f")
            nc.vector.reciprocal(rec_f[:], sums_sb[:])
            rec = nrmp.tile([4, S], F32R, name="rec")
            nc.vector.tensor_copy(rec[:], rec_f[:])
            onorm = [nrmp.tile([128, S], PJ_DT, name=f"onorm{m}") for m in range(2)]
            for m in range(2):
                for qt in range(QT):
                    qsl = bass.ts(qt, 512)
                    bc = pbcp.tile([128, 512], F32, tag="bc", name="bc")
                    nc.tensor.matmul(bc[:], sel_sb[:, m * 128:(m + 1) * 128],
                                     rec[:, qsl], start=True, stop=True)
                    nc.vector.tensor_mul(onorm[m][:, qsl], o_sb[m][:, qsl], bc[:])
            for mo in range(HT):
                for t in range(QT):
                    tsl = bass.ts(t, 512)
                    yp = pyp.tile([128, 512], F32, tag="yp", name="yp")
                    for k2 in range(2):
                        nc.tensor.matmul(yp[:], wo_sb[:, k2, mo * 128:(mo + 1) * 128],
                                         onorm[k2][:, tsl],
                                         start=(k2 == 0), stop=(k2 == 1))
                    ysb = ysbp.tile([128, 512], F32, tag="ysb", name="ysb")
                    nc.vector.tensor_copy(ysb[:], yp[:])
                    nc.scalar.dma_start(out=d["yT"][mo * 128:(mo + 1) * 128, tsl],
                                        in_=ysb[:])


def build(reps=1, split=True, phases=4):
    nc = bass.Bass("TRN2", target_bir_lowering=False, debug=False,
                   num_devices=N_CORES)
    pj = F32 if PJ_DT == F32R else PJ_DT
    sc = F32 if SC_DT == F32R else SC_DT
    d = {
        "xT": nc.dram_tensor("xT", [H, S], pj, kind="ExternalInput"),
        "wqkT": nc.dram_tensor("wqkT", [H, 512], pj, kind="ExternalInput"),
        "wvT": nc.dram_tensor("wvT", [H, 260], pj, kind="ExternalInput"),
        "woT": nc.dram_tensor("woT", [DL, H], pj, kind="ExternalInput"),
        "cosT": nc.dram_tensor("cosT", [128, S], F32, kind="ExternalInput"),
        "sinT": nc.dram_tensor("sinT", [128, S], F32, kind="ExternalInput"),
        "perm": nc.dram_tensor("perm", [128, 128], sc, kind="ExternalInput"),
        "sel": nc.dram_tensor("sel", [4, 256], F32, kind="ExternalInput"),
        "mrow": nc.dram_tensor("mrow", [1, S], sc, kind="ExternalInput"),
        "ones_row": nc.dram_tensor("ones_row", [1, S], sc, kind="ExternalInput"),
        "yT": nc.dram_tensor("yT", [H, S], F32, kind="ExternalOutput"),
    }
    with tile.TileContext(nc) as tc:
        if reps == 1:
            _emit_body(nc, tc, d, phases)
        else:
            with tc.For_i(0, reps, 1):
                _emit_body(nc, tc, d, phases)
    if split:
        _split_multiwait(nc)
    return nc


def host_inputs(x, attention_mask, Wq, Wk, Wv, Wo):
    """Build the 8 per-core input maps (numpy only)."""
    x = np.asarray(x, dtype=np.float32)
    attention_mask = np.asarray(attention_mask, dtype=np.float32)
    Wq = np.asarray(Wq, dtype=np.float32)
    Wk = np.asarray(Wk, dtype=np.float32)
    Wv = np.asarray(Wv, dtype=np.float32)
    Wo = np.asarray(Wo, dtype=np.float32)

    pj_np = _np_dt[PJ_DT]
    sc_np = _np_dt[SC_DT]

    xT = [np.ascontiguousarray(x[b].T).astype(pj_np) for b in range(B)]

    p = np.arange(128)
    dd = p % HD
    inv = ROPE_BASE ** (-(dd % 32).astype(np.float32) / 32.0)
    s = np.arange(S, dtype=np.float32)
    ang = inv[:, None] * s[None, :]
    cosT = np.cos(ang).astype(np.float32)
    sinT = (np.where(dd < 32, -1.0, 1.0)[:, None] * np.sin(ang)).astype(np.float32)

    perm = np.zeros((128, 128), dtype=sc_np)
    for m in range(128):
        head, d_ = m // HD, m % HD
        perm[head * HD + (d_ + 32) % HD, m] = 1.0

    sel = np.zeros((4, 256), dtype=np.float32)
    for m in range(256):
        sel[m // HD, m] = 1.0

    ones_row = np.ones((1, S), dtype=sc_np)
    mrow = [(MASK_NEG * (1.0 - attention_mask[b])).astype(sc_np).reshape(1, S)
            for b in range(B)]

    in_maps = []
    for c in range(N_CORES):
        b, hq = c // 4, (c % 4) * HPC
        d0 = hq * HD
        wqkT = np.ascontiguousarray(
            np.concatenate([Wq[d0:d0 + DL] * (HD ** -0.5), Wk[d0:d0 + DL]], axis=0).T
        ).astype(pj_np)
        wvT = np.zeros((H, 260), dtype=pj_np)
        for hh in range(HPC):
            wvT[:, hh * 65:hh * 65 + 64] = Wv[d0 + hh * HD:d0 + (hh + 1) * HD].T
        woT = np.ascontiguousarray(Wo[:, d0:d0 + DL].T).astype(pj_np)
        in_maps.append({
            "xT": xT[b], "wqkT": wqkT, "wvT": wvT, "woT": woT,
            "cosT": cosT, "sinT": sinT, "perm": perm, "sel": sel,
            "mrow": mrow[b], "ones_row": ones_row,
        })
    return in_maps


def gather_output(results):
    y = np.zeros((B, S, H), dtype=np.float32)
    for c in range(N_CORES):
        y[c // 4] += results[c]["yT"].T
    return y


_nc_cache = {}


def kernel(x, attention_mask, Wq, Wk, Wv, Wo):
    if "nc" not in _nc_cache:
        _nc_cache["nc"] = build(reps=1)
    nc = _nc_cache["nc"]
    in_maps = host_inputs(x, attention_mask, Wq, Wk, Wv, Wo)
    res = run_bass_kernel_spmd(nc, in_maps, list(range(N_CORES)), trace=False)
    return gather_output(res.results)
